# revision 9
# baseline (speedup 1.0000x reference)
"""EnhancedGCN (3-layer GCN + BN + ReLU) on 8 Trainium2 NeuronCores.

Strategy:
  - Nodes partitioned across 8 cores by dst range (graph parallel).
  - Sequential split packing: each core's node segments (edge lists sorted
    by dst) are laid out densely across 128-slot tiles; a segment may span
    two tiles (never a region) and PSUM accumulation merges the halves, so
    slot fill is ~0.998. Tile t of a region structurally owns columns
    [4t, 4t+4) for segments STARTING in it (<=4 starts/tile) and its matmul
    writes the 8-wide window [4t-4, 4t+4) to also cover a continuation from
    tile t-1. This keeps the SPMD program identical across cores; all
    per-core variation lives in the idx/mask input tensors.
  - One canonical gather-index table serves all 3 layers (layer 1's table is
    x pre-permuted and pre-scaled by dinv host-side). Tables and masks are
    bf16; mask values (dinv[dst]) are dithered between the two neighboring
    bf16 values so each node's slot-sum is unbiased.
  - Device per region (128 tiles, 512 psum columns): zero acc via a
    zero-lhsT matmul, then per tile ONE indirect DMA gathers 128 rows (the
    SWDGE per-instruction limit) and one PE matmul (lhsT = gathered
    [128,cin], rhs = mask [128,w<=8]) accumulates.
  - Epilogue per region: W matmul (f32r), BN partial stats, stash f32.
    After layers 1-2: AllReduce BN stats, fused BN+ReLU, scale by
    dinv[node], PE transpose to node-major, AllGather the bf16 table.
    Layer 3 applies W3+b3 and writes the per-core output slice.
Host unpermutes canonical column order back to natural node order.
"""
import numpy as np

N_NODES = 100000
N_CORES = 8
EPS = 1e-5
TILE = 128        # slots per tile
RTILES = 128      # tiles per region (=> 512 columns)
MAXW = 8          # mask window width per tile


def _bf16(a):
    import ml_dtypes
    return np.asarray(a, dtype=ml_dtypes.bfloat16)


# ----------------------------------------------------------------- host plan
def _pack_core(degs):
    """Sequential split packing. Returns (ntiles, col_of_node, starts) where
    col_of_node[i] is the region-column id (4*t_local + m) of node i and
    slot layout is implicit: segments in node order, padding where noted.
    Returns slot fill plan as arrays: for each node, (tile, pos) of its
    segment start, plus per-tile padding info via the layout walk."""
    n = len(degs)
    tile = 0
    pos = 0
    starts = 0
    col = np.full(n, -1, np.int64)
    seg_tile = np.zeros(n, np.int64)
    seg_pos = np.zeros(n, np.int64)
    region_left = RTILES
    for i in range(n):
        d = int(degs[i])
        rem_region = (region_left - 1) * TILE + (TILE - pos)
        if d > rem_region:
            # pad to region end
            tile += region_left
            pos = 0
            starts = 0
            region_left = RTILES
        if starts >= 4:
            tile += 1
            pos = 0
            starts = 0
            region_left -= 1
            if region_left == 0:
                region_left = RTILES
            rem_region = (region_left - 1) * TILE + (TILE - pos)
            if d > rem_region:
                tile += region_left
                pos = 0
                region_left = RTILES
        t_local = (tile % RTILES)
        col[i] = (tile // RTILES) * 512 + 4 * t_local + starts
        seg_tile[i] = tile
        seg_pos[i] = pos
        starts += 1
        pos += d
        while pos >= TILE:
            pos -= TILE
            tile += 1
            starts = 0
            region_left -= 1
            if region_left == 0:
                region_left = RTILES
    ntiles = tile + (1 if pos > 0 else 0)
    return ntiles, col, seg_tile, seg_pos


def _plan(edge_index, n_nodes, n_cores):
    src = np.asarray(edge_index[0], np.int64)
    dst = np.asarray(edge_index[1], np.int64)
    deg = np.bincount(dst, minlength=n_nodes).astype(np.int64)  # slot count/node
    deg_full = deg + 1                                          # incl self-loop
    assert deg.max() <= TILE, f"degree {deg.max()} exceeds {TILE}"
    dinv = (1.0 / np.sqrt(np.maximum(deg_full, 1.0))).astype(np.float32)

    # dithered bf16 neighbors of dinv
    u = dinv.view(np.uint32)
    qlo_bits = u & np.uint32(0xFFFF0000)
    qlo = qlo_bits.view(np.float32)
    qhi = (qlo_bits + np.uint32(0x10000)).view(np.float32)
    denom = np.maximum(qhi - qlo, 1e-30)
    frac = np.clip((dinv - qlo) / denom, 0.0, 1.0)
    m_hi = np.round(deg * frac).astype(np.int64)      # slots using qhi (no loop)

    per = n_nodes // n_cores
    order = np.argsort(dst, kind="stable")
    src_s = src[order]
    starts_e = np.searchsorted(dst[order], np.arange(n_nodes))

    packs = []
    max_tiles = 0
    for c in range(n_cores):
        degs = deg[c * per:(c + 1) * per]
        nt, col, seg_tile, seg_pos = _pack_core(degs)
        packs.append((col, seg_tile, seg_pos))
        max_tiles = max(max_tiles, nt)
    T = ((max_tiles + RTILES - 1) // RTILES) * RTILES
    R = T // RTILES
    QC = 4 * T

    canon = np.zeros(n_nodes, np.int64)
    idx = np.zeros((n_cores, 128, T), np.int32)
    masks = np.zeros((n_cores, 128, MAXW * T), np.float32)
    dcol_pp = np.zeros((n_cores, 128, 4 * R), np.float32)
    node_of_col = np.full((n_cores, QC), -1, np.int64)

    for c in range(n_cores):
        lo = c * per
        col, seg_tile, seg_pos = packs[c]
        nodes = np.arange(lo, lo + per)
        canon[nodes] = c * QC + col
        node_of_col[c, col] = nodes
        dv = np.zeros(QC, np.float32)
        dv[col] = dinv[nodes]
        dcol_pp[c] = dv.reshape(4 * R, 128).T
        # per-edge slot placement (vectorized)
        degs = deg[lo:lo + per]
        E = int(degs.sum())
        gpos = seg_tile * TILE + seg_pos          # global slot of segment start
        slot = np.repeat(gpos, degs) + (np.arange(E) -
                                        np.repeat(np.cumsum(degs) - degs, degs))
        kk = np.arange(E) - np.repeat(np.cumsum(degs) - degs, degs)
        nd = np.repeat(np.arange(per), degs)      # node (local) per slot
        tl = slot // TILE
        pp = slot % TILE
        srcs = np.concatenate([src_s[starts_e[n]:starts_e[n] + degs[n - lo]]
                               for n in nodes]) if False else None
        # gather srcs per node in node order: dst-sorted src_s is contiguous
        # per node, but nodes are contiguous too -> one slice per core? No:
        # starts_e[n] ranges are contiguous across consecutive n.
        s0 = starts_e[lo]
        srcs = src_s[s0:s0 + E]
        # mask value: window coord = col - (4*t_local - 4), t_local = tl%RTILES
        t_local = tl % RTILES
        wcol = (col[nd] % 512) - (4 * t_local - 4)
        assert (wcol >= 0).all() and (wcol < MAXW).all()
        val = np.where(kk < m_hi[lo:lo + per][nd], qhi[lo:lo + per][nd],
                       qlo[lo:lo + per][nd])
        idx[c, pp, tl] = srcs.astype(np.int32)
        masks[c, pp, MAXW * tl + wcol] = val
    idxg = np.zeros_like(idx)
    for c in range(n_cores):
        idxg[c] = canon[idx[c].reshape(-1).astype(np.int64)].reshape(128, T)
    return dict(dinv=dinv, idx=idxg.astype(np.int32), masks=masks,
                dcol_pp=dcol_pp, node_of_col=node_of_col, canon=canon,
                T=T, R=R, QC=QC, per=per)


def make_in_maps(plan, inputs, n_nodes):
    """Per-core input dicts for the built kernel."""
    QC = plan["QC"]
    NG = N_CORES * QC
    x = np.asarray(inputs["x"], np.float32)
    xc = np.zeros((NG, 8), np.float32)
    for c in range(N_CORES):
        noc = plan["node_of_col"][c]
        m = noc >= 0
        rows = c * QC + np.nonzero(m)[0]
        xc[rows, :5] = x[noc[m]] * plan["dinv"][noc[m]][:, None]
    xc = _bf16(xc)
    W1p = np.zeros((8, 32), np.float32)
    W1p[:5] = np.asarray(inputs["W1"], np.float32)
    gb = np.stack([inputs["gamma1"], inputs["beta1"],
                   inputs["gamma2"], inputs["beta2"]], axis=1).astype(np.float32)
    eye = np.eye(128, dtype=np.float32)
    maps = []
    for c in range(N_CORES):
        maps.append({
            "x_c": xc,
            "x_loc": xc[c * QC:(c + 1) * QC],
            "idx": plan["idx"][c],
            "masks": _bf16(plan["masks"][c]),
            "dcol": plan["dcol_pp"][c],
            "W1p": W1p,
            "W2": np.asarray(inputs["W2"], np.float32),
            "W3": np.asarray(inputs["W3"], np.float32),
            "gb": gb,
            "b3": np.asarray(inputs["b3"], np.float32).reshape(1, 1),
            "eye32": eye,
        })
    return maps


def unpermute(plan, results, n_nodes):
    out = np.zeros((n_nodes, 1), np.float32)
    for c in range(N_CORES):
        vals = np.asarray(results[c]["out"], np.float32).reshape(-1)
        noc = plan["node_of_col"][c]
        m = noc >= 0
        out[noc[m], 0] = vals[:len(noc)][m]
    return out


# --------------------------------------------------------------- device build
def _build(plan, n_nodes, n_cores):
    import concourse.bacc as bacc
    import concourse.bass as bass
    import concourse.mybir as mybir
    import concourse.tile as tile

    f32 = mybir.dt.float32
    f32r = mybir.dt.float32r
    bf16 = mybir.dt.bfloat16
    T, R, QC = plan["T"], plan["R"], plan["QC"]
    NG = n_cores * QC
    ds = bass.ds

    nc = bacc.Bacc("TRN2", target_bir_lowering=False, debug=False,
                   num_devices=n_cores)
    # ---- inputs
    x_c = nc.dram_tensor("x_c", [NG, 8], bf16, kind="ExternalInput")
    x_loc = nc.dram_tensor("x_loc", [QC, 8], bf16, kind="ExternalInput")
    idx_d = nc.dram_tensor("idx", [128, T], mybir.dt.int32, kind="ExternalInput")
    masks_d = nc.dram_tensor("masks", [128, MAXW * T], bf16, kind="ExternalInput")
    dcol_d = nc.dram_tensor("dcol", [128, 4 * R], f32, kind="ExternalInput")
    w1_d = nc.dram_tensor("W1p", [8, 32], f32, kind="ExternalInput")
    w2_d = nc.dram_tensor("W2", [32, 32], f32, kind="ExternalInput")
    w3_d = nc.dram_tensor("W3", [32, 1], f32, kind="ExternalInput")
    gb_d = nc.dram_tensor("gb", [32, 4], f32, kind="ExternalInput")
    b3_d = nc.dram_tensor("b3", [1, 1], f32, kind="ExternalInput")
    eye_d = nc.dram_tensor("eye32", [128, 128], f32, kind="ExternalInput")
    out_d = nc.dram_tensor("out", [1, QC], f32, kind="ExternalOutput")

    RG = list(range(n_cores))
    SROWS = ((R + 3) // 4) * 512

    with tile.TileContext(nc) as tc:
        with (
            tc.tile_pool(name="const", bufs=1) as cpool,
            tc.tile_pool(name="sb", bufs=2) as pool,
            tc.tile_pool(name="mbuf", bufs=2) as mpool,
            tc.tile_pool(name="gbuf", bufs=2) as gpool,
            tc.tile_pool(name="ps", bufs=2, space="PSUM") as psum,
            tc.tile_pool(name="ps2", bufs=2, space="PSUM") as psum2,
            tc.tile_pool(name="ps3", bufs=2, space="PSUM") as psum3,
            tc.tile_pool(name="dram", bufs=1, space="DRAM") as dpool,
        ):
            w1_f = cpool.tile([8, 32], f32)
            nc.sync.dma_start(out=w1_f[:], in_=w1_d[:, :])
            w1_t = cpool.tile([8, 32], f32r)
            nc.scalar.copy(out=w1_t[:], in_=w1_f[:])
            w2_f = cpool.tile([32, 32], f32)
            nc.sync.dma_start(out=w2_f[:], in_=w2_d[:, :])
            w2_t = cpool.tile([32, 32], f32r)
            nc.scalar.copy(out=w2_t[:], in_=w2_f[:])
            w3_f = cpool.tile([32, 1], f32)
            nc.sync.dma_start(out=w3_f[:], in_=w3_d[:, :])
            w3_t = cpool.tile([32, 1], f32r)
            nc.scalar.copy(out=w3_t[:], in_=w3_f[:])
            gb_t = cpool.tile([32, 4], f32)
            nc.sync.dma_start(out=gb_t[:], in_=gb_d[:, :])
            b3_t = cpool.tile([1, 1], f32)
            nc.sync.dma_start(out=b3_t[:], in_=b3_d[:, :])
            eye_t = cpool.tile([128, 128], f32)
            nc.sync.dma_start(out=eye_t[:], in_=eye_d[:, :])
            eps_t = cpool.tile([32, 1], f32)
            nc.vector.memset(eps_t[:], float(EPS))
            idx_sb = cpool.tile([128, T], mybir.dt.int32)
            nc.sync.dma_start(out=idx_sb[:], in_=idx_d[:, :])
            dcol_sb = cpool.tile([128, 4 * R], f32)
            nc.sync.dma_start(out=dcol_sb[:], in_=dcol_d[:, :])
            stash = cpool.tile([128, SROWS], f32)
            zlhs = cpool.tile([128, 32], bf16)
            nc.vector.memset(zlhs[:], 0.0)
            zrhs = cpool.tile([128, 512], bf16)
            nc.vector.memset(zrhs[:], 0.0)

            # ---- tables (DRAM)
            t2loc = dpool.tile([QC, 32], bf16, name="t2loc")
            t2glob = dpool.tile([NG, 32], bf16, name="t2glob",
                                addr_space="Shared")
            t3loc = dpool.tile([QC, 32], bf16, name="t3loc")
            t3glob = dpool.tile([NG, 32], bf16, name="t3glob",
                                addr_space="Shared")

            def agg_layer(table_ap, loc_ap, cin, stats_acc=None, wt=None,
                          l3=False):
                for r in range(R):
                    mreg = mpool.tile([128, MAXW * RTILES], bf16, tag="mreg",
                                      name="mreg")
                    nc.sync.dma_start(
                        out=mreg[:],
                        in_=masks_d[:, r * MAXW * RTILES:(r + 1) * MAXW * RTILES])
                    g = gpool.tile([128, RTILES * cin], bf16, tag=f"g{cin}",
                                   name=f"g{cin}")
                    acc = psum.tile([cin, 512], f32, tag="acc", name="acc")
                    nc.tensor.matmul(acc[:], zlhs[:, :cin], zrhs[:],
                                     start=True, stop=False,
                                     skip_group_check=True)
                    tls = pool.tile([128, 4 * cin], bf16, tag="tls",
                                    name="tls")
                    nc.sync.dma_start(
                        out=tls[:].rearrange("p (k c) -> p k c", k=4),
                        in_=loc_ap[r * 512:(r + 1) * 512, :].rearrange(
                            "(k p) c -> p k c", p=128))
                    for k in range(4):
                        sc = pool.tile([128, cin], f32, tag="sct", name="sc")
                        nc.scalar.mul(out=sc[:],
                                      in_=tls[:, ds(cin * k, cin)],
                                      mul=dcol_sb[:, 4 * r + k:4 * r + k + 1])
                        nc.tensor.matmul(
                            acc[:, ds(128 * k, 128)], sc[:], eye_t[:, :],
                            is_transpose=True, start=False, stop=False,
                            skip_group_check=True)
                    for t in range(RTILES):
                        tg = r * RTILES + t
                        nc.gpsimd.indirect_dma_start(
                            out=g[:, ds(cin * t, cin)], out_offset=None,
                            in_=table_ap,
                            in_offset=bass.IndirectOffsetOnAxis(
                                ap=idx_sb[:, ds(tg, 1)], axis=0))
                        w0 = max(4 * t - 4, 0)
                        w = 4 * t + 4 - w0
                        nc.tensor.matmul(
                            acc[:, ds(w0, w)], g[:, ds(cin * t, cin)],
                            mreg[:, ds(MAXW * t + (MAXW - w), w)],
                            start=False, stop=False, skip_group_check=True)
                    agg_sb = pool.tile([cin, 512], f32r, tag="aggsb",
                                       name="agg_sb")
                    nc.scalar.copy(out=agg_sb[:], in_=acc[:])
                    if not l3:
                        hps = psum2.tile([32, 512], f32, tag="hps", name="hps")
                        nc.tensor.matmul(hps[:], wt[:], agg_sb[:],
                                         start=True, stop=True,
                                         skip_group_check=True)
                        s1 = pool.tile([32, 1], f32, tag="s1t", name="s1")
                        nc.vector.reduce_sum(out=s1[:], in_=hps[:],
                                             axis=mybir.AxisListType.X)
                        sq = pool.tile([32, 512], f32, tag="sqt", name="sq")
                        nc.scalar.square(out=sq[:], in_=hps[:])
                        s2 = pool.tile([32, 1], f32, tag="s2t", name="s2")
                        nc.vector.reduce_sum(out=s2[:], in_=sq[:],
                                             axis=mybir.AxisListType.X)
                        nc.vector.tensor_add(out=stats_acc[:, 0:1],
                                             in0=stats_acc[:, 0:1], in1=s1[:])
                        nc.vector.tensor_add(out=stats_acc[:, 1:2],
                                             in0=stats_acc[:, 1:2], in1=s2[:])
                        sl = stash[32 * (r % 4):32 * (r % 4) + 32,
                                   512 * (r // 4):512 * (r // 4) + 512]
                        nc.scalar.copy(out=sl, in_=hps[:])
                    else:
                        ops = psum2.tile([1, 512], f32, tag="ops", name="ops")
                        nc.tensor.matmul(ops[:], wt[:], agg_sb[:],
                                         start=True, stop=True,
                                         skip_group_check=True)
                        ot = pool.tile([1, 512], f32, tag="ot", name="ot")
                        nc.scalar.activation(
                            out=ot[:], in_=ops[:],
                            func=mybir.ActivationFunctionType.Identity,
                            bias=b3_t[:, 0:1], scale=1.0)
                        nc.sync.dma_start(out=out_d[:, r * 512:(r + 1) * 512],
                                          in_=ot[:])

            def bn_pass(stats_acc, gi, tloc, tglob, inv_n):
                sin = dpool.tile([32, 2], f32, name=f"sin{gi}")
                sout = dpool.tile([32, 2], f32, name=f"sout{gi}",
                                  addr_space="Shared")
                nc.sync.dma_start(out=sin[:, :], in_=stats_acc[:])
                nc.gpsimd.collective_compute(
                    "AllReduce", mybir.AluOpType.add, replica_groups=[RG],
                    ins=[sin[:, :].opt()], outs=[sout[:, :].opt()])
                st = pool.tile([32, 2], f32, tag="stt", name="st")
                nc.sync.dma_start(out=st[:], in_=sout[:, :])
                mean = pool.tile([32, 1], f32, tag="bn1", name="mean")
                nc.scalar.mul(out=mean[:], in_=st[:, 0:1], mul=inv_n)
                ex2 = pool.tile([32, 1], f32, tag="bn2", name="ex2")
                nc.scalar.mul(out=ex2[:], in_=st[:, 1:2], mul=inv_n)
                m2 = pool.tile([32, 1], f32, tag="bn3", name="m2")
                nc.scalar.square(out=m2[:], in_=mean[:])
                var = pool.tile([32, 1], f32, tag="bn4", name="var")
                nc.vector.tensor_tensor(out=var[:], in0=ex2[:], in1=m2[:],
                                        op=mybir.AluOpType.subtract)
                sd = pool.tile([32, 1], f32, tag="bn5", name="sd")
                nc.scalar.activation(out=sd[:], in_=var[:],
                                     func=mybir.ActivationFunctionType.Sqrt,
                                     bias=eps_t[:, 0:1], scale=1.0)
                inv = pool.tile([32, 1], f32, tag="bn6", name="inv")
                nc.vector.reciprocal(out=inv[:], in_=sd[:])
                A = pool.tile([32, 1], f32, tag="bn7", name="A")
                nc.vector.tensor_mul(out=A[:], in0=gb_t[:, 2 * gi:2 * gi + 1],
                                     in1=inv[:])
                mA = pool.tile([32, 1], f32, tag="bn8", name="mA")
                nc.vector.tensor_mul(out=mA[:], in0=mean[:], in1=A[:])
                B = pool.tile([32, 1], f32, tag="bn9", name="B")
                nc.vector.tensor_tensor(out=B[:], in0=gb_t[:, 2 * gi + 1:2 * gi + 2],
                                        in1=mA[:], op=mybir.AluOpType.subtract)
                for r in range(R):
                    sl = stash[32 * (r % 4):32 * (r % 4) + 32,
                               512 * (r // 4):512 * (r // 4) + 512]
                    un = pool.tile([32, 512], f32, tag="un", name="un")
                    nc.scalar.activation(out=un[:], in_=sl,
                                         func=mybir.ActivationFunctionType.Relu,
                                         bias=B[:, 0:1], scale=A[:, 0:1])
                    tsb = pool.tile([128, 4 * 32], bf16, tag="tsb", name="tsb")
                    for k in range(4):
                        tp = psum3.tile([128, 32], f32, tag="tp", name="tp")
                        nc.tensor.transpose(tp[:], un[:, 128 * k:128 * k + 128],
                                            eye_t[:32, :32])
                        nc.scalar.mul(out=tsb[:, 32 * k:32 * k + 32], in_=tp[:],
                                      mul=dcol_sb[:, 4 * r + k:4 * r + k + 1])
                    nc.sync.dma_start(
                        out=tloc[r * 512:(r + 1) * 512, :].rearrange(
                            "(k p) c -> p k c", p=128),
                        in_=tsb[:].rearrange("p (k c) -> p k c", k=4))
                nc.gpsimd.collective_compute(
                    "AllGather", mybir.AluOpType.bypass, replica_groups=[RG],
                    ins=[tloc[:, :].opt()], outs=[tglob[:, :].opt()])

            # L1
            stats1 = cpool.tile([32, 2], f32)
            nc.vector.memset(stats1[:], 0.0)
            agg_layer(x_c[:, :], x_loc[:, :], 8, stats_acc=stats1,
                      wt=w1_t)
            bn_pass(stats1, 0, t2loc, t2glob, 1.0 / n_nodes)
            # L2
            stats2 = cpool.tile([32, 2], f32)
            nc.vector.memset(stats2[:], 0.0)
            agg_layer(t2glob[:, :], t2loc[:, :], 32, stats_acc=stats2,
                      wt=w2_t)
            bn_pass(stats2, 1, t3loc, t3glob, 1.0 / n_nodes)
            # L3
            agg_layer(t3glob[:, :], t3loc[:, :], 32, wt=w3_t, l3=True)

    nc.compile()
    return nc


# ------------------------------------------------------------------- kernel
def kernel(x, edge_index, W1, b1, gamma1, beta1, W2, b2, gamma2, beta2, W3, b3):
    from concourse.bass_utils import run_bass_kernel_spmd

    inputs = dict(x=x, edge_index=edge_index, W1=W1, b1=b1, gamma1=gamma1,
                  beta1=beta1, W2=W2, b2=b2, gamma2=gamma2, beta2=beta2,
                  W3=W3, b3=b3)
    x = np.asarray(x, np.float32)
    n_nodes = x.shape[0]
    plan = _plan(np.asarray(edge_index), n_nodes, N_CORES)
    nc = _build(plan, n_nodes, N_CORES)
    in_maps = make_in_maps(plan, inputs, n_nodes)
    res = run_bass_kernel_spmd(nc, in_maps, core_ids=list(range(N_CORES)))
    return unpermute(plan, res.results, n_nodes)


# revision 10
# speedup vs baseline: 1.0197x; 1.0197x over previous
"""EnhancedGCN (3-layer GCN + BN + ReLU) on 8 Trainium2 NeuronCores.

Strategy:
  - Nodes partitioned across 8 cores by dst range (graph parallel).
  - Sequential split packing: each core's node segments (edge lists sorted
    by dst) are laid out densely across 128-slot tiles; a segment may span
    two tiles (never a region) and PSUM accumulation merges the halves, so
    slot fill is ~0.998. Tile t of a region structurally owns columns
    [4t, 4t+4) for segments STARTING in it (<=4 starts/tile) and its matmul
    writes the 8-wide window [4t-4, 4t+4) to also cover a continuation from
    tile t-1. This keeps the SPMD program identical across cores; all
    per-core variation lives in the idx/mask input tensors.
  - One canonical gather-index table serves all 3 layers (layer 1's table is
    x pre-permuted and pre-scaled by dinv host-side). Tables and masks are
    bf16; mask values (dinv[dst]) are dithered between the two neighboring
    bf16 values so each node's slot-sum is unbiased.
  - Device per region (128 tiles, 512 psum columns): zero acc via a
    zero-lhsT matmul, then per tile ONE indirect DMA gathers 128 rows (the
    SWDGE per-instruction limit) and one PE matmul (lhsT = gathered
    [128,cin], rhs = mask [128,w<=8]) accumulates.
  - Epilogue per region: W matmul (f32r), BN partial stats, stash f32.
    After layers 1-2: AllReduce BN stats, fused BN+ReLU, scale by
    dinv[node], PE transpose to node-major, AllGather the bf16 table.
    Layer 3 applies W3+b3 and writes the per-core output slice.
Host unpermutes canonical column order back to natural node order.
"""
import numpy as np

N_NODES = 100000
N_CORES = 8
EPS = 1e-5
TILE = 128        # slots per tile
RTILES = 128      # tiles per region (=> 512 columns)
MAXW = 8          # mask window width per tile


def _bf16(a):
    import ml_dtypes
    return np.asarray(a, dtype=ml_dtypes.bfloat16)


# ----------------------------------------------------------------- host plan
def _pack_core(degs):
    """Sequential split packing. Returns (ntiles, col_of_node, starts) where
    col_of_node[i] is the region-column id (4*t_local + m) of node i and
    slot layout is implicit: segments in node order, padding where noted.
    Returns slot fill plan as arrays: for each node, (tile, pos) of its
    segment start, plus per-tile padding info via the layout walk."""
    n = len(degs)
    tile = 0
    pos = 0
    starts = 0
    col = np.full(n, -1, np.int64)
    seg_tile = np.zeros(n, np.int64)
    seg_pos = np.zeros(n, np.int64)
    region_left = RTILES
    for i in range(n):
        d = int(degs[i])
        rem_region = (region_left - 1) * TILE + (TILE - pos)
        if d > rem_region:
            # pad to region end
            tile += region_left
            pos = 0
            starts = 0
            region_left = RTILES
        if starts >= 4:
            tile += 1
            pos = 0
            starts = 0
            region_left -= 1
            if region_left == 0:
                region_left = RTILES
            rem_region = (region_left - 1) * TILE + (TILE - pos)
            if d > rem_region:
                tile += region_left
                pos = 0
                region_left = RTILES
        t_local = (tile % RTILES)
        col[i] = (tile // RTILES) * 512 + 4 * t_local + starts
        seg_tile[i] = tile
        seg_pos[i] = pos
        starts += 1
        pos += d
        while pos >= TILE:
            pos -= TILE
            tile += 1
            starts = 0
            region_left -= 1
            if region_left == 0:
                region_left = RTILES
    ntiles = tile + (1 if pos > 0 else 0)
    return ntiles, col, seg_tile, seg_pos


def _plan(edge_index, n_nodes, n_cores):
    src = np.asarray(edge_index[0], np.int64)
    dst = np.asarray(edge_index[1], np.int64)
    deg = np.bincount(dst, minlength=n_nodes).astype(np.int64)  # slot count/node
    deg_full = deg + 1                                          # incl self-loop
    assert deg.max() <= TILE, f"degree {deg.max()} exceeds {TILE}"
    dinv = (1.0 / np.sqrt(np.maximum(deg_full, 1.0))).astype(np.float32)

    # dithered bf16 neighbors of dinv
    u = dinv.view(np.uint32)
    qlo_bits = u & np.uint32(0xFFFF0000)
    qlo = qlo_bits.view(np.float32)
    qhi = (qlo_bits + np.uint32(0x10000)).view(np.float32)
    denom = np.maximum(qhi - qlo, 1e-30)
    frac = np.clip((dinv - qlo) / denom, 0.0, 1.0)
    m_hi = np.round(deg * frac).astype(np.int64)      # slots using qhi (no loop)

    per = n_nodes // n_cores
    order = np.argsort(dst, kind="stable")
    src_s = src[order]
    starts_e = np.searchsorted(dst[order], np.arange(n_nodes))

    packs = []
    max_tiles = 0
    for c in range(n_cores):
        degs = deg[c * per:(c + 1) * per]
        nt, col, seg_tile, seg_pos = _pack_core(degs)
        packs.append((col, seg_tile, seg_pos))
        max_tiles = max(max_tiles, nt)
    T = ((max_tiles + RTILES - 1) // RTILES) * RTILES
    R = T // RTILES
    NT = max_tiles
    QC = 4 * T

    canon = np.zeros(n_nodes, np.int64)
    idx = np.zeros((n_cores, 128, T), np.int32)
    masks = np.zeros((n_cores, 128, MAXW * T), np.float32)
    dcol_pp = np.zeros((n_cores, 128, 4 * R), np.float32)
    node_of_col = np.full((n_cores, QC), -1, np.int64)

    for c in range(n_cores):
        lo = c * per
        col, seg_tile, seg_pos = packs[c]
        nodes = np.arange(lo, lo + per)
        canon[nodes] = c * QC + col
        node_of_col[c, col] = nodes
        dv = np.zeros(QC, np.float32)
        dv[col] = dinv[nodes]
        dcol_pp[c] = dv.reshape(4 * R, 128).T
        # per-edge slot placement (vectorized)
        degs = deg[lo:lo + per]
        E = int(degs.sum())
        gpos = seg_tile * TILE + seg_pos          # global slot of segment start
        slot = np.repeat(gpos, degs) + (np.arange(E) -
                                        np.repeat(np.cumsum(degs) - degs, degs))
        kk = np.arange(E) - np.repeat(np.cumsum(degs) - degs, degs)
        nd = np.repeat(np.arange(per), degs)      # node (local) per slot
        tl = slot // TILE
        pp = slot % TILE
        srcs = np.concatenate([src_s[starts_e[n]:starts_e[n] + degs[n - lo]]
                               for n in nodes]) if False else None
        # gather srcs per node in node order: dst-sorted src_s is contiguous
        # per node, but nodes are contiguous too -> one slice per core? No:
        # starts_e[n] ranges are contiguous across consecutive n.
        s0 = starts_e[lo]
        srcs = src_s[s0:s0 + E]
        # mask value: window coord = col - (4*t_local - 4), t_local = tl%RTILES
        t_local = tl % RTILES
        wcol = (col[nd] % 512) - (4 * t_local - 4)
        assert (wcol >= 0).all() and (wcol < MAXW).all()
        val = np.where(kk < m_hi[lo:lo + per][nd], qhi[lo:lo + per][nd],
                       qlo[lo:lo + per][nd])
        idx[c, pp, tl] = srcs.astype(np.int32)
        masks[c, pp, MAXW * tl + wcol] = val
    idxg = np.zeros_like(idx)
    for c in range(n_cores):
        idxg[c] = canon[idx[c].reshape(-1).astype(np.int64)].reshape(128, T)
    return dict(dinv=dinv, idx=idxg.astype(np.int32), masks=masks,
                dcol_pp=dcol_pp, node_of_col=node_of_col, canon=canon,
                T=T, R=R, NT=NT, QC=QC, per=per)


def make_in_maps(plan, inputs, n_nodes):
    """Per-core input dicts for the built kernel."""
    QC = plan["QC"]
    NG = N_CORES * QC
    x = np.asarray(inputs["x"], np.float32)
    xc = np.zeros((NG, 8), np.float32)
    for c in range(N_CORES):
        noc = plan["node_of_col"][c]
        m = noc >= 0
        rows = c * QC + np.nonzero(m)[0]
        xc[rows, :5] = x[noc[m]] * plan["dinv"][noc[m]][:, None]
    xc = _bf16(xc)
    W1p = np.zeros((8, 32), np.float32)
    W1p[:5] = np.asarray(inputs["W1"], np.float32)
    gb = np.stack([inputs["gamma1"], inputs["beta1"],
                   inputs["gamma2"], inputs["beta2"]], axis=1).astype(np.float32)
    eye = np.eye(128, dtype=np.float32)
    maps = []
    for c in range(N_CORES):
        maps.append({
            "x_c": xc,
            "x_loc": xc[c * QC:(c + 1) * QC],
            "idx": plan["idx"][c],
            "masks": _bf16(plan["masks"][c]),
            "dcol": plan["dcol_pp"][c],
            "W1p": W1p,
            "W2": np.asarray(inputs["W2"], np.float32),
            "W3": np.asarray(inputs["W3"], np.float32),
            "gb": gb,
            "b3": np.asarray(inputs["b3"], np.float32).reshape(1, 1),
            "eye32": eye,
        })
    return maps


def unpermute(plan, results, n_nodes):
    out = np.zeros((n_nodes, 1), np.float32)
    for c in range(N_CORES):
        vals = np.asarray(results[c]["out"], np.float32).reshape(-1)
        noc = plan["node_of_col"][c]
        m = noc >= 0
        out[noc[m], 0] = vals[:len(noc)][m]
    return out


# --------------------------------------------------------------- device build
def _build(plan, n_nodes, n_cores):
    import concourse.bacc as bacc
    import concourse.bass as bass
    import concourse.mybir as mybir
    import concourse.tile as tile

    f32 = mybir.dt.float32
    f32r = mybir.dt.float32r
    bf16 = mybir.dt.bfloat16
    T, R, QC = plan["T"], plan["R"], plan["QC"]
    NT = plan.get("NT", T)
    NG = n_cores * QC
    ds = bass.ds

    nc = bacc.Bacc("TRN2", target_bir_lowering=False, debug=False,
                   num_devices=n_cores)
    # ---- inputs
    x_c = nc.dram_tensor("x_c", [NG, 8], bf16, kind="ExternalInput")
    x_loc = nc.dram_tensor("x_loc", [QC, 8], bf16, kind="ExternalInput")
    idx_d = nc.dram_tensor("idx", [128, T], mybir.dt.int32, kind="ExternalInput")
    masks_d = nc.dram_tensor("masks", [128, MAXW * T], bf16, kind="ExternalInput")
    dcol_d = nc.dram_tensor("dcol", [128, 4 * R], f32, kind="ExternalInput")
    w1_d = nc.dram_tensor("W1p", [8, 32], f32, kind="ExternalInput")
    w2_d = nc.dram_tensor("W2", [32, 32], f32, kind="ExternalInput")
    w3_d = nc.dram_tensor("W3", [32, 1], f32, kind="ExternalInput")
    gb_d = nc.dram_tensor("gb", [32, 4], f32, kind="ExternalInput")
    b3_d = nc.dram_tensor("b3", [1, 1], f32, kind="ExternalInput")
    eye_d = nc.dram_tensor("eye32", [128, 128], f32, kind="ExternalInput")
    out_d = nc.dram_tensor("out", [1, QC], f32, kind="ExternalOutput")

    RG = list(range(n_cores))
    SROWS = ((R + 3) // 4) * 512

    with tile.TileContext(nc) as tc:
        with (
            tc.tile_pool(name="const", bufs=1) as cpool,
            tc.tile_pool(name="sb", bufs=2) as pool,
            tc.tile_pool(name="mbuf", bufs=2) as mpool,
            tc.tile_pool(name="gbuf", bufs=2) as gpool,
            tc.tile_pool(name="ps", bufs=2, space="PSUM") as psum,
            tc.tile_pool(name="ps2", bufs=2, space="PSUM") as psum2,
            tc.tile_pool(name="ps3", bufs=2, space="PSUM") as psum3,
            tc.tile_pool(name="dram", bufs=1, space="DRAM") as dpool,
        ):
            w1_f = cpool.tile([8, 32], f32)
            nc.sync.dma_start(out=w1_f[:], in_=w1_d[:, :])
            w1_t = cpool.tile([8, 32], f32r)
            nc.scalar.copy(out=w1_t[:], in_=w1_f[:])
            w2_f = cpool.tile([32, 32], f32)
            nc.sync.dma_start(out=w2_f[:], in_=w2_d[:, :])
            w2_t = cpool.tile([32, 32], f32r)
            nc.scalar.copy(out=w2_t[:], in_=w2_f[:])
            w3_f = cpool.tile([32, 1], f32)
            nc.sync.dma_start(out=w3_f[:], in_=w3_d[:, :])
            w3_t = cpool.tile([32, 1], f32r)
            nc.scalar.copy(out=w3_t[:], in_=w3_f[:])
            gb_t = cpool.tile([32, 4], f32)
            nc.sync.dma_start(out=gb_t[:], in_=gb_d[:, :])
            b3_t = cpool.tile([1, 1], f32)
            nc.sync.dma_start(out=b3_t[:], in_=b3_d[:, :])
            eye_t = cpool.tile([128, 128], f32)
            nc.sync.dma_start(out=eye_t[:], in_=eye_d[:, :])
            eps_t = cpool.tile([32, 1], f32)
            nc.vector.memset(eps_t[:], float(EPS))
            idx_sb = cpool.tile([128, T], mybir.dt.int32)
            nc.sync.dma_start(out=idx_sb[:], in_=idx_d[:, :])
            dcol_sb = cpool.tile([128, 4 * R], f32)
            nc.sync.dma_start(out=dcol_sb[:], in_=dcol_d[:, :])
            stash = cpool.tile([128, SROWS], f32)
            zlhs = cpool.tile([128, 32], bf16)
            nc.vector.memset(zlhs[:], 0.0)
            zrhs = cpool.tile([128, 512], bf16)
            nc.vector.memset(zrhs[:], 0.0)

            # ---- tables (DRAM)
            t2loc = dpool.tile([QC, 32], bf16, name="t2loc")
            t2glob = dpool.tile([NG, 32], bf16, name="t2glob",
                                addr_space="Shared")
            t3loc = dpool.tile([QC, 32], bf16, name="t3loc")
            t3glob = dpool.tile([NG, 32], bf16, name="t3glob",
                                addr_space="Shared")

            def agg_layer(table_ap, loc_ap, cin, stats_acc=None, wt=None,
                          l3=False):
                for r in range(R):
                    mreg = mpool.tile([128, MAXW * RTILES], bf16, tag="mreg",
                                      name="mreg")
                    nc.sync.dma_start(
                        out=mreg[:],
                        in_=masks_d[:, r * MAXW * RTILES:(r + 1) * MAXW * RTILES])
                    g = gpool.tile([128, RTILES * cin], bf16, tag=f"g{cin}",
                                   name=f"g{cin}")
                    acc = psum.tile([cin, 512], f32, tag="acc", name="acc")
                    nc.tensor.matmul(acc[:], zlhs[:, :cin], zrhs[:],
                                     start=True, stop=False,
                                     skip_group_check=True)
                    tls = pool.tile([128, 4 * cin], bf16, tag="tls",
                                    name="tls")
                    nc.sync.dma_start(
                        out=tls[:].rearrange("p (k c) -> p k c", k=4),
                        in_=loc_ap[r * 512:(r + 1) * 512, :].rearrange(
                            "(k p) c -> p k c", p=128))
                    for k in range(4):
                        sc = pool.tile([128, cin], f32, tag="sct", name="sc")
                        nc.scalar.mul(out=sc[:],
                                      in_=tls[:, ds(cin * k, cin)],
                                      mul=dcol_sb[:, 4 * r + k:4 * r + k + 1])
                        nc.tensor.matmul(
                            acc[:, ds(128 * k, 128)], sc[:], eye_t[:, :],
                            is_transpose=True, start=False, stop=False,
                            skip_group_check=True)
                    nt_r = min(RTILES, NT - r * RTILES)
                    for t in range(nt_r):
                        tg = r * RTILES + t
                        nc.gpsimd.indirect_dma_start(
                            out=g[:, ds(cin * t, cin)], out_offset=None,
                            in_=table_ap,
                            in_offset=bass.IndirectOffsetOnAxis(
                                ap=idx_sb[:, ds(tg, 1)], axis=0))
                        w0 = max(4 * t - 4, 0)
                        w = 4 * t + 4 - w0
                        nc.tensor.matmul(
                            acc[:, ds(w0, w)], g[:, ds(cin * t, cin)],
                            mreg[:, ds(MAXW * t + (MAXW - w), w)],
                            start=False, stop=False, skip_group_check=True)
                    agg_sb = pool.tile([cin, 512], f32r, tag="aggsb",
                                       name="agg_sb")
                    nc.scalar.copy(out=agg_sb[:], in_=acc[:])
                    if not l3:
                        hps = psum2.tile([32, 512], f32, tag="hps", name="hps")
                        nc.tensor.matmul(hps[:], wt[:], agg_sb[:],
                                         start=True, stop=True,
                                         skip_group_check=True)
                        s1 = pool.tile([32, 1], f32, tag="s1t", name="s1")
                        nc.vector.reduce_sum(out=s1[:], in_=hps[:],
                                             axis=mybir.AxisListType.X)
                        sq = pool.tile([32, 512], f32, tag="sqt", name="sq")
                        nc.scalar.square(out=sq[:], in_=hps[:])
                        s2 = pool.tile([32, 1], f32, tag="s2t", name="s2")
                        nc.vector.reduce_sum(out=s2[:], in_=sq[:],
                                             axis=mybir.AxisListType.X)
                        nc.vector.tensor_add(out=stats_acc[:, 0:1],
                                             in0=stats_acc[:, 0:1], in1=s1[:])
                        nc.vector.tensor_add(out=stats_acc[:, 1:2],
                                             in0=stats_acc[:, 1:2], in1=s2[:])
                        sl = stash[32 * (r % 4):32 * (r % 4) + 32,
                                   512 * (r // 4):512 * (r // 4) + 512]
                        nc.scalar.copy(out=sl, in_=hps[:])
                    else:
                        ops = psum2.tile([1, 512], f32, tag="ops", name="ops")
                        nc.tensor.matmul(ops[:], wt[:], agg_sb[:],
                                         start=True, stop=True,
                                         skip_group_check=True)
                        ot = pool.tile([1, 512], f32, tag="ot", name="ot")
                        nc.scalar.activation(
                            out=ot[:], in_=ops[:],
                            func=mybir.ActivationFunctionType.Identity,
                            bias=b3_t[:, 0:1], scale=1.0)
                        nc.sync.dma_start(out=out_d[:, r * 512:(r + 1) * 512],
                                          in_=ot[:])

            def bn_pass(stats_acc, gi, tloc, tglob, inv_n):
                sin = dpool.tile([32, 2], f32, name=f"sin{gi}")
                sout = dpool.tile([32, 2], f32, name=f"sout{gi}",
                                  addr_space="Shared")
                nc.sync.dma_start(out=sin[:, :], in_=stats_acc[:])
                nc.gpsimd.collective_compute(
                    "AllReduce", mybir.AluOpType.add, replica_groups=[RG],
                    ins=[sin[:, :].opt()], outs=[sout[:, :].opt()])
                st = pool.tile([32, 2], f32, tag="stt", name="st")
                nc.sync.dma_start(out=st[:], in_=sout[:, :])
                mean = pool.tile([32, 1], f32, tag="bn1", name="mean")
                nc.scalar.mul(out=mean[:], in_=st[:, 0:1], mul=inv_n)
                ex2 = pool.tile([32, 1], f32, tag="bn2", name="ex2")
                nc.scalar.mul(out=ex2[:], in_=st[:, 1:2], mul=inv_n)
                m2 = pool.tile([32, 1], f32, tag="bn3", name="m2")
                nc.scalar.square(out=m2[:], in_=mean[:])
                var = pool.tile([32, 1], f32, tag="bn4", name="var")
                nc.vector.tensor_tensor(out=var[:], in0=ex2[:], in1=m2[:],
                                        op=mybir.AluOpType.subtract)
                sd = pool.tile([32, 1], f32, tag="bn5", name="sd")
                nc.scalar.activation(out=sd[:], in_=var[:],
                                     func=mybir.ActivationFunctionType.Sqrt,
                                     bias=eps_t[:, 0:1], scale=1.0)
                inv = pool.tile([32, 1], f32, tag="bn6", name="inv")
                nc.vector.reciprocal(out=inv[:], in_=sd[:])
                A = pool.tile([32, 1], f32, tag="bn7", name="A")
                nc.vector.tensor_mul(out=A[:], in0=gb_t[:, 2 * gi:2 * gi + 1],
                                     in1=inv[:])
                mA = pool.tile([32, 1], f32, tag="bn8", name="mA")
                nc.vector.tensor_mul(out=mA[:], in0=mean[:], in1=A[:])
                B = pool.tile([32, 1], f32, tag="bn9", name="B")
                nc.vector.tensor_tensor(out=B[:], in0=gb_t[:, 2 * gi + 1:2 * gi + 2],
                                        in1=mA[:], op=mybir.AluOpType.subtract)
                for r in range(R):
                    sl = stash[32 * (r % 4):32 * (r % 4) + 32,
                               512 * (r // 4):512 * (r // 4) + 512]
                    un = pool.tile([32, 512], f32, tag="un", name="un")
                    nc.scalar.activation(out=un[:], in_=sl,
                                         func=mybir.ActivationFunctionType.Relu,
                                         bias=B[:, 0:1], scale=A[:, 0:1])
                    tsb = pool.tile([128, 4 * 32], bf16, tag="tsb", name="tsb")
                    for k in range(4):
                        tp = psum3.tile([128, 32], f32, tag="tp", name="tp")
                        nc.tensor.transpose(tp[:], un[:, 128 * k:128 * k + 128],
                                            eye_t[:32, :32])
                        nc.scalar.mul(out=tsb[:, 32 * k:32 * k + 32], in_=tp[:],
                                      mul=dcol_sb[:, 4 * r + k:4 * r + k + 1])
                    nc.sync.dma_start(
                        out=tloc[r * 512:(r + 1) * 512, :].rearrange(
                            "(k p) c -> p k c", p=128),
                        in_=tsb[:].rearrange("p (k c) -> p k c", k=4))
                nc.gpsimd.collective_compute(
                    "AllGather", mybir.AluOpType.bypass, replica_groups=[RG],
                    ins=[tloc[:, :].opt()], outs=[tglob[:, :].opt()])

            # L1
            stats1 = cpool.tile([32, 2], f32)
            nc.vector.memset(stats1[:], 0.0)
            agg_layer(x_c[:, :], x_loc[:, :], 8, stats_acc=stats1,
                      wt=w1_t)
            bn_pass(stats1, 0, t2loc, t2glob, 1.0 / n_nodes)
            # L2
            stats2 = cpool.tile([32, 2], f32)
            nc.vector.memset(stats2[:], 0.0)
            agg_layer(t2glob[:, :], t2loc[:, :], 32, stats_acc=stats2,
                      wt=w2_t)
            bn_pass(stats2, 1, t3loc, t3glob, 1.0 / n_nodes)
            # L3
            agg_layer(t3glob[:, :], t3loc[:, :], 32, wt=w3_t, l3=True)

    nc.compile()
    return nc


# ------------------------------------------------------------------- kernel
def kernel(x, edge_index, W1, b1, gamma1, beta1, W2, b2, gamma2, beta2, W3, b3):
    from concourse.bass_utils import run_bass_kernel_spmd

    inputs = dict(x=x, edge_index=edge_index, W1=W1, b1=b1, gamma1=gamma1,
                  beta1=beta1, W2=W2, b2=b2, gamma2=gamma2, beta2=beta2,
                  W3=W3, b3=b3)
    x = np.asarray(x, np.float32)
    n_nodes = x.shape[0]
    plan = _plan(np.asarray(edge_index), n_nodes, N_CORES)
    nc = _build(plan, n_nodes, N_CORES)
    in_maps = make_in_maps(plan, inputs, n_nodes)
    res = run_bass_kernel_spmd(nc, in_maps, core_ids=list(range(N_CORES)))
    return unpermute(plan, res.results, n_nodes)
